# revision 1
# baseline (speedup 1.0000x reference)
"""Trainium2 Bass kernel for nn_DiscreteDecisionEngine.

Math: the reference computes
    q = tanh(geodesic_weights)            # [1, N, 4], N = 256
    h = L(q) (x)  (quaternion Hamilton product per 4-group)
    logits = h_flat @ W.T + b
The Hamilton product is a block-diagonal (4x4 per group) linear map B(q)
applied to x, so logits = x @ (W @ B)^T + b. We fold W' = W @ B on the
host (tiny: [256,1024] weights) and run a pure GEMM on 8 NeuronCores,
data-parallel over the batch.

Device kernel per core (x shard [8192, 1024] f32), DMA-stream-bound:
  for each group of 2 row-tiles (1 MB DMA in, on the SP HWDGE ring):
    per 128-row tile: PE-transpose 128x128 chunks (fp32, 4 per PSUM bank),
    DVE cast-copy -> fp32r (TF32) SBUF, 8 accumulating fp32r matmuls
    psum[128b, 256a] += xT_k.T @ W'T_k, DVE bias-add fused with copyback
    group store [128, 2, 256] via the ACT HWDGE ring
  (software-pipelined one group ahead; last 4 tiles emitted solo to
  shorten the drain)
"""

import os
from contextlib import ExitStack

import numpy as np

import concourse.bass as bass
import concourse.mybir as mybir
import concourse.tile as tile
from concourse import bacc
from concourse.bass import ts
from concourse.bass_utils import run_bass_kernel_spmd
from concourse.masks import make_identity

N_CORES = 8
B_FULL = 65536
B_SHARD = B_FULL // N_CORES  # 8192
D = 1024
A = 256  # num actions
KC = D // 128  # 8 contraction chunks

_F32 = mybir.dt.float32
_F32R = mybir.dt.float32r
_F16 = mybir.dt.float16

# tuning knobs (overridable via env for A/B experiments)
_ACT_COPY_BANK = int(os.environ.get("K_ACT_COPY_BANK", "-1"))
_PIPE = int(os.environ.get("K_PIPE", "1"))
_GROUP = int(os.environ.get("K_GROUP", "2"))  # batch tiles per DMA
_OUT_ON_ACT = bool(int(os.environ.get("K_OUT_ON_ACT", "1")))
_FIRST_SPLIT = int(os.environ.get("K_FIRST_SPLIT", "1024"))  # cols of first sub-load
_TAIL_SPLIT = int(os.environ.get("K_TAIL_SPLIT", "6"))  # trailing tiles emitted solo
_IN_ALT_RING = bool(int(os.environ.get("K_IN_ALT_RING", "0")))
_W_FP16 = bool(int(os.environ.get("K_W_FP16", "1")))  # ship W' as fp16 (exact in TF32)
_TAIL_COLSPLIT = int(os.environ.get("K_TAIL_COLSPLIT", "0"))  # tail groups w/ split loads
_HEAD_SPLIT = int(os.environ.get("K_HEAD_SPLIT", "0"))  # leading tiles emitted solo
_TAIL_ACT = bool(int(os.environ.get("K_TAIL_ACT", "1")))  # ACT copyback in the drain
_DRAIN_FINE = bool(int(os.environ.get("K_DRAIN_FINE", "0")))  # 2-chunk drain copies
_DRAIN_STORE_SP = bool(int(os.environ.get("K_DRAIN_STORE_SP", "1")))  # drain stores on SP ring
_HOLD_STORES = int(os.environ.get("K_HOLD_STORES", "0"))  # early groups' stores deferred to drain
_X16 = bool(int(os.environ.get("K_X16", "0")))  # cast x to fp16, fp16 transpose+matmul
_X16_DVE_MOD = int(os.environ.get("K_X16_DVE_MOD", "2"))  # every Nth group casts on DVE
_BUFS_XIN = int(os.environ.get("K_BUFS_XIN", "5"))
_BUFS_TP = int(os.environ.get("K_BUFS_TP", "4"))
_BUFS_XT = int(os.environ.get("K_BUFS_XT", "4"))
_BUFS_PO = int(os.environ.get("K_BUFS_PO", "3"))
_BUFS_OB = int(os.environ.get("K_BUFS_OB", "4"))


def _build_nc():
    nc = bacc.Bacc(None, target_bir_lowering=False)

    x = nc.dram_tensor("x", [B_SHARD, D], _F32, kind="ExternalInput")
    # w[p, k*A + a] = W'[a, 128*k + p]  (host-prepared, SBUF layout).
    # fp16 halves the transfer; its 11-bit significand matches TF32, so the
    # device-side upconvert to f32r is exact for these magnitudes.
    w = nc.dram_tensor("w", [128, KC * A], _F16 if _W_FP16 else _F32R,
                       kind="ExternalInput")
    # bias broadcast to all 128 partitions on host
    bias = nc.dram_tensor("bias", [128, A], _F32, kind="ExternalInput")
    out = nc.dram_tensor("out", [B_SHARD, A], _F32, kind="ExternalOutput")

    with ExitStack() as ctx:
        tc = ctx.enter_context(tile.TileContext(nc))
        const = ctx.enter_context(tc.tile_pool(name="const", bufs=1))

        xin = ctx.enter_context(tc.tile_pool(name="xin", bufs=_BUFS_XIN))
        tp = ctx.enter_context(tc.tile_pool(name="tp", bufs=_BUFS_TP, space="PSUM"))
        xt = ctx.enter_context(tc.tile_pool(name="xt", bufs=_BUFS_XT))
        po = ctx.enter_context(tc.tile_pool(name="po", bufs=_BUFS_PO, space="PSUM"))
        ob = ctx.enter_context(tc.tile_pool(name="ob", bufs=_BUFS_OB))
        obh = (
            ctx.enter_context(tc.tile_pool(name="obh", bufs=_HOLD_STORES))
            if _HOLD_STORES > 0
            else None
        )

        n_tiles = B_SHARD // 128
        G = _GROUP
        # schedule of (first_tile, group_size); head/tail split into
        # single-tile groups to start the PE earlier / shorten the drain
        head = min(_HEAD_SPLIT, n_tiles)
        tail = min(_TAIL_SPLIT, n_tiles - head)
        main_tiles = n_tiles - head - tail
        assert main_tiles % G == 0
        sched = [(j, 1) for j in range(head)]
        sched += [(head + i * G, G) for i in range(main_tiles // G)]
        sched += [(head + main_tiles + j, 1) for j in range(tail)]
        n_groups = len(sched)
        staged = {}

        # first x load is issued before the (1MB) weight load so the PE's
        # transposes start as early as possible; ident is device-generated
        ident = const.tile([128, 128], _F32)
        make_identity(nc, ident)
        g0 = sched[0][1]
        xg0 = xin.tile([128, g0, D], _F32, tag=f"xg{g0}")
        src0 = x[bass.ds(0, g0 * 128), :]
        if g0 > 1:
            src0 = src0.rearrange("(t p) d -> p t d", p=128)
        else:
            src0 = src0.rearrange("p (t d) -> p t d", t=1)
        nc.sync.dma_start(xg0[:, 0, ts(0, _FIRST_SPLIT)], src0[:, 0, ts(0, _FIRST_SPLIT)])
        if _FIRST_SPLIT < D:
            nc.sync.dma_start(
                xg0[:, 0, _FIRST_SPLIT:], src0[:, 0, _FIRST_SPLIT:]
            )
        for t in range(1, g0):
            nc.sync.dma_start(xg0[:, t, :], src0[:, t, :])

        # weights/bias ride the ACT HWDGE ring (idle at startup) so they
        # don't delay the x stream on the SP ring
        if _X16:
            # matmul consumes fp16 weights directly; drain tiles stay on the
            # f32r path (no cast stage in their latency chain), so keep both
            w16 = const.tile([128, KC, A], _F16)
            nc.scalar.dma_start(w16[:], w.rearrange("p (k a) -> p k a", k=KC))
            w_sb = const.tile([128, KC, A], _F32R)
            nc.vector.tensor_copy(out=w_sb[:], in_=w16[:])
            w_mm = w_sb
            ident16 = const.tile([128, 128], _F16)
            make_identity(nc, ident16)
        elif _W_FP16:
            w_sb = const.tile([128, KC, A], _F32R)
            w16 = const.tile([128, KC, A], _F16)
            nc.scalar.dma_start(w16[:], w.rearrange("p (k a) -> p k a", k=KC))
            nc.vector.tensor_copy(out=w_sb[:], in_=w16[:])
            w_mm = w_sb
        else:
            w_sb = const.tile([128, KC, A], _F32R)
            nc.scalar.dma_start(w_sb[:], w.rearrange("p (k a) -> p k a", k=KC))
            w_mm = w_sb
        bias_sb = const.tile([128, A], _F32)
        nc.scalar.dma_start(bias_sb[:], bias[:])

        def stage_load_transpose(gi):
            row0, g = sched[gi]
            if gi == 0:
                xg = xg0
            else:
                xg = xin.tile([128, g, D], _F32, tag=f"xg{g}")
                src = x[ts(row0, 128) if g == 1 else bass.ds(row0 * 128, g * 128), :]
                if g > 1:
                    src = src.rearrange("(t p) d -> p t d", p=128)
                else:
                    src = src.rearrange("p (t d) -> p t d", t=1)
                if _IN_ALT_RING and gi % 2 == 1:
                    nc.scalar.dma_start(xg[:], src)
                elif g == 1 and gi >= n_groups - _TAIL_COLSPLIT:
                    # split the last loads by column halves so the drain's
                    # transposes start before the full tile lands
                    nc.sync.dma_start(xg[:, :, : D // 2], src[:, :, : D // 2])
                    nc.sync.dma_start(xg[:, :, D // 2 :], src[:, :, D // 2 :])
                else:
                    nc.sync.dma_start(xg[:], src)
            xts = []
            in_drain = _TAIL_ACT and row0 >= n_tiles - _TAIL_SPLIT
            use16 = _X16 and not in_drain
            if use16:
                # cast the group to fp16 (11-bit significand, same as TF32's)
                # on ACT/DVE before the PE transposes; halves PE transpose and
                # DVE copyback time
                xg16 = xin.tile([128, g, D], _F16, tag=f"x16{g}")
                cast_eng = (
                    nc.vector.tensor_copy
                    if (_X16_DVE_MOD > 0 and gi % _X16_DVE_MOD == 0)
                    else nc.scalar.copy
                )
                for t in range(g):
                    cast_eng(out=xg16[:, t, :], in_=xg[:, t, :])
                xg = xg16
            t_ident = ident16 if use16 else ident
            t_dt = _F16 if use16 else _F32
            xt_dt = _F16 if use16 else _F32R
            if in_drain and _DRAIN_FINE:
                for t in range(g):
                    xt_tile = xt.tile([128, KC, 128], xt_dt, tag="xt")
                    for h in range(KC // 2):
                        pt = tp.tile([128, 2, 128], t_dt, tag="pt")
                        for j in range(2):
                            k = 2 * h + j
                            nc.tensor.transpose(
                                pt[:, j, :], xg[:, t, ts(k, 128)], t_ident[:]
                            )
                        if h % 2 == 1:
                            nc.scalar.copy(out=xt_tile[:, ts(h, 2), :], in_=pt[:])
                        else:
                            nc.vector.tensor_copy(
                                out=xt_tile[:, ts(h, 2), :], in_=pt[:]
                            )
                    xts.append(xt_tile)
                staged[gi] = (xts, use16)
                return
            for t in range(g):
                xt_tile = xt.tile([128, KC, 128], xt_dt, tag="xt")
                for g in range(KC // 4):
                    # 4 transposed chunks per PSUM bank -> single wide copyback
                    pt = tp.tile([128, 4, 128], t_dt, tag="pt")
                    for j in range(4):
                        k = 4 * g + j
                        nc.tensor.transpose(
                            pt[:, j, :], xg[:, t, ts(k, 128)], t_ident[:]
                        )
                    # cast-copy f32 -> f32r (TF32 rounding) for the PE;
                    # optionally alternate banks between DVE and ACT
                    in_drain = _TAIL_ACT and row0 >= n_tiles - _TAIL_SPLIT
                    if (_ACT_COPY_BANK >= 0 and g % 2 == _ACT_COPY_BANK) or (
                        in_drain and g % 2 == 1
                    ):
                        nc.scalar.copy(out=xt_tile[:, ts(g, 4), :], in_=pt[:])
                    else:
                        nc.vector.tensor_copy(out=xt_tile[:, ts(g, 4), :], in_=pt[:])
                xts.append(xt_tile)
            staged[gi] = (xts, use16)

        held_stores = []

        def stage_matmul_store(gi):
            row0, g = sched[gi]
            xts, use16 = staged.pop(gi)
            hold = gi < _HOLD_STORES
            if hold:
                og = obh.tile([128, g, A], _F32, tag=f"obh{g}")
            else:
                og = ob.tile([128, g, A], _F32, tag=f"ob{g}")
            for t in range(g):
                p_out = po.tile([128, A], _F32)
                for k in range(KC):
                    nc.tensor.matmul(
                        p_out[:],
                        lhsT=xts[t][:, k, :],
                        rhs=(w16 if use16 else w_mm)[:, k, :],
                        start=(k == 0),
                        stop=(k == KC - 1),
                    )
                nc.vector.tensor_add(og[:, t, :], p_out[:], bias_sb[:])
            dst = out[bass.ds(row0 * 128, g * 128), :]
            if g > 1:
                dst = dst.rearrange("(t p) a -> p t a", p=128)
            else:
                dst = dst.rearrange("p (t a) -> p t a", t=1)
            if hold:
                # store deferred: flushed right before the drain groups so the
                # in-stream finishes earlier and these fill the drain window
                held_stores.append((dst, og))
                return
            drain_store_sp = _DRAIN_STORE_SP and row0 >= n_tiles - _TAIL_SPLIT
            if _OUT_ON_ACT and not drain_store_sp:
                nc.scalar.dma_start(dst, og[:])
            else:
                nc.sync.dma_start(dst, og[:])

        # optional software pipeline: emit transposes of group i+PIPE before
        # matmuls of group i
        first_drain = n_groups - tail
        for i in range(n_groups + _PIPE):
            if i == first_drain and held_stores:
                for dst_h, og_h in held_stores:
                    nc.scalar.dma_start(dst_h, og_h[:])
                held_stores.clear()
            if i < n_groups:
                stage_load_transpose(i)
            if i >= _PIPE:
                stage_matmul_store(i - _PIPE)

    nc.finalize()  # runs Bacc.compile(): wait-splitting etc.
    return nc


_NC_CACHE = None
LAST_RESULTS = None


def _get_nc():
    global _NC_CACHE
    if _NC_CACHE is None:
        _NC_CACHE = _build_nc()
    return _NC_CACHE


def _fold_weights(geodesic_weights: np.ndarray, W: np.ndarray) -> np.ndarray:
    """W' = W @ blockdiag(L(tanh(g))^T per 4-group), in float64."""
    q = np.tanh(geodesic_weights.astype(np.float64))[0]  # [N, 4]
    w_, i_, j_, k_ = q[:, 0], q[:, 1], q[:, 2], q[:, 3]
    n = q.shape[0]
    M = np.empty((n, 4, 4), dtype=np.float64)  # y_r = sum_s M[n, r, s] x_s
    M[:, 0] = np.stack([w_, -i_, -j_, -k_], axis=-1)
    M[:, 1] = np.stack([i_, w_, -k_, j_], axis=-1)
    M[:, 2] = np.stack([j_, k_, w_, -i_], axis=-1)
    M[:, 3] = np.stack([k_, -j_, i_, w_], axis=-1)
    W4 = W.astype(np.float64).reshape(A, n, 4)  # [a, n, r]
    Wp = np.einsum("anr,nrs->ans", W4, M).reshape(A, D)
    return Wp.astype(np.float32)  # [a, d]


def kernel(x, geodesic_weights, W, b, **_unused):
    x = np.ascontiguousarray(np.asarray(x, dtype=np.float32))
    Wp = _fold_weights(np.asarray(geodesic_weights), np.asarray(W))
    # device layout: w_dev[p, k*A + a] = Wp[a, 128k + p]
    w_dev = np.ascontiguousarray(
        Wp.T.reshape(KC, 128, A).transpose(1, 0, 2).reshape(128, KC * A)
    )
    if _W_FP16:
        w_dev = w_dev.astype(np.float16)
    bias_dev = np.ascontiguousarray(
        np.broadcast_to(np.asarray(b, dtype=np.float32)[None, :], (128, A))
    )

    nc = _get_nc()
    shards = np.split(x, N_CORES, axis=0)
    in_maps = [{"x": s, "w": w_dev, "bias": bias_dev} for s in shards]
    res = run_bass_kernel_spmd(
        nc,
        in_maps,
        core_ids=list(range(N_CORES)),
        trace=bool(int(os.environ.get("KERNEL_TRACE", "0"))),
    )
    global LAST_RESULTS
    LAST_RESULTS = res
    out = np.concatenate([r["out"] for r in res.results], axis=0)
    return out



# revision 31
# speedup vs baseline: 1.9322x; 1.9322x over previous
"""Trainium2 Bass kernel for nn_DiscreteDecisionEngine.

Math: the reference computes
    q = tanh(geodesic_weights)            # [1, N, 4], N = 256
    h = L(q) (x)  (quaternion Hamilton product per 4-group)
    logits = h_flat @ W.T + b
The Hamilton product is a block-diagonal (4x4 per group) linear map B(q)
applied to x, so logits = x @ (W @ B)^T + b. We fold W' = W @ B on the
host (tiny: [256,1024] weights) and run a pure GEMM on 8 NeuronCores,
data-parallel over the batch.

All device DMAs serialize at the ~360 GB/s per-core HBM roofline, so the
kernel ships every tensor in fp16 (11-bit significand keeps ~2^-11
relative accuracy, far inside the 2e-2 gate): x is cast AND pre-tiled/
transposed on host so the contraction dim lands on partitions with 2 KB
contiguous DMA lines, W' ships fp16, and logits leave the device as fp16
(upcast on host). Per-core traffic: 16.78 MB in + 4.19 MB out + 0.52 MB
weights ~= 21.5 MB ~= 59.7 us of DMA at the roofline (the all-f32
variant moves 42.6 MB).

Device per 128-row tile: 8 accumulating fp16 matmuls -> PSUM f32
[128b, 256a]; DVE adds bias (PE-broadcast from a [1,256] row at startup)
fused with the fp16 cast; loads stream on the SP HWDGE ring, stores ride
the ACT ring. Early store groups are held in SBUF and flushed during the
drain so the DMA engines never idle while the last tiles' compute
finishes; the final tiles load solo (last one split in column halves) to
shorten the dependence tail.
"""

import os
from contextlib import ExitStack

import numpy as np

import concourse.bass as bass
import concourse.mybir as mybir
import concourse.tile as tile
from concourse import bacc
from concourse.bass import ts
from concourse.bass_utils import run_bass_kernel_spmd

N_CORES = 8
B_FULL = 65536
B_SHARD = B_FULL // N_CORES  # 8192
D = 1024
A = 256  # num actions
KC = D // 128  # 8 contraction chunks

_F16 = mybir.dt.float16
_F32 = mybir.dt.float32

# tuning knobs (overridable via env for A/B experiments)
_G = int(os.environ.get("K_G", "2"))  # tiles per load group (main stream)
_TAIL = int(os.environ.get("K_TAIL", "4"))  # trailing tiles loaded solo
_TAIL_SPLIT_LAST = int(os.environ.get("K_TAIL_SPLIT_LAST", "1"))  # split last tile load
_HOLD0 = int(os.environ.get("K_HOLD0", "26"))  # first held tile (held: HOLD0..59)
_FLUSH_CHUNK = int(os.environ.get("K_FLUSH_CHUNK", "4"))  # tiles per flush DMA
_PIPE = int(os.environ.get("K_PIPE", "1"))  # groups of load lookahead
_GS_DRAIN_SP = int(os.environ.get("K_GS_DRAIN_SP", "1"))  # drain stores ride SP ring
_BUFS_XIN = int(os.environ.get("K_BUFS_XIN", "6"))
_BUFS_PO = int(os.environ.get("K_BUFS_PO", "5"))
_BUFS_OB = int(os.environ.get("K_BUFS_OB", "6"))


def _build_nc():
    nc = bacc.Bacc(None, target_bir_lowering=False)

    # x_dev[t*128 + p, k*128 + b] = x_f16[t*128 + b, k*128 + p]
    # (host-pretransposed per-tile: contraction dim on partitions, 2 KB
    # contiguous per partition line per tile)
    x = nc.dram_tensor("x", [B_SHARD, KC * 128], _F16, kind="ExternalInput")
    # w[p, k*A + a] = W'[a, 128*k + p]  (host-prepared SBUF layout, fp16)
    w = nc.dram_tensor("w", [128, KC * A], _F16, kind="ExternalInput")
    bias = nc.dram_tensor("bias", [128, A], _F16, kind="ExternalInput")
    out = nc.dram_tensor("out", [B_SHARD, A], _F16, kind="ExternalOutput")

    with ExitStack() as ctx:
        tc = ctx.enter_context(tile.TileContext(nc))
        const = ctx.enter_context(tc.tile_pool(name="const", bufs=1))
        xin = ctx.enter_context(tc.tile_pool(name="xin", bufs=_BUFS_XIN))
        po = ctx.enter_context(tc.tile_pool(name="po", bufs=_BUFS_PO, space="PSUM"))
        pob = ctx.enter_context(tc.tile_pool(name="pob", bufs=1, space="PSUM"))
        poh = ctx.enter_context(tc.tile_pool(name="poh", bufs=2, space="PSUM"))
        ob = ctx.enter_context(tc.tile_pool(name="ob", bufs=_BUFS_OB))
        obh = ctx.enter_context(tc.tile_pool(name="obh", bufs=1))

        n_tiles = B_SHARD // 128
        tail = min(_TAIL, n_tiles)
        main_tiles = n_tiles - tail
        hold0 = min(_HOLD0, main_tiles)
        n_held = main_tiles - hold0  # tiles hold0..main_tiles-1 held in SBUF
        assert main_tiles % _G == 0
        sched = [(i * _G, _G) for i in range(main_tiles // _G)]
        sched += [(main_tiles + j, 1) for j in range(tail)]
        n_groups = len(sched)
        staged = {}

        def load_src(row0, g):
            src = x[ts(row0, 128) if g == 1 else bass.ds(row0 * 128, g * 128), :]
            if g > 1:
                return src.rearrange("(t p) c -> p t c", p=128)
            return src.rearrange("p (t c) -> p t c", t=1)

        # first x group rides SP immediately, then the weights in k-halves
        # (also SP, so their transfers pipeline right behind g0 without the
        # ACT ring's slower issue path); the tiny host-broadcast bias rides
        # ACT. Group 0's matmuls are emitted k-half-major below so the PE
        # starts on w's first half a full transfer earlier.
        g0 = sched[0][1]
        xg0 = xin.tile([128, g0, KC * 128], _F16, tag=f"xg{g0}")
        nc.sync.dma_start(xg0[:], load_src(0, g0))

        w_sb = const.tile([128, KC, A], _F16)
        w_src = w.rearrange("p (k a) -> p k a", k=KC)
        w_split = int(os.environ.get("K_W_SPLIT", "2"))
        for c in range(w_split):
            k0, k1 = c * KC // w_split, (c + 1) * KC // w_split
            nc.sync.dma_start(w_sb[:, k0:k1, :], w_src[:, k0:k1, :])
        bias_sb = const.tile([128, A], _F16)
        nc.scalar.dma_start(bias_sb[:], bias[:])

        ones = const.tile([1, A], _F16)
        nc.gpsimd.memset(ones[:], 1.0)
        # dummy matmuls anchor the PE pstate-ramp clock right after the
        # startup barrier, so the real matmuls (first load lands ~5.8 us in)
        # hit full speed almost immediately
        ps_b = pob.tile([128, A], _F32, tag="ps_b")
        for _ in range(4):
            nc.tensor.matmul(ps_b[:], lhsT=ones[:, :128], rhs=ones[:], start=True, stop=True)

        def stage_load(gi):
            row0, g = sched[gi]
            if gi == 0:
                staged[gi] = xg0
                return
            xg = xin.tile([128, g, KC * 128], _F16, tag=f"xg{g}")
            src = load_src(row0, g)
            if g == 1 and _TAIL_SPLIT_LAST and gi == n_groups - 1:
                # split the last load by column (=k-chunk) halves so its
                # first 4 matmuls overlap the second half's transfer
                H = KC * 128 // 2
                nc.sync.dma_start(xg[:, :, :H], src[:, :, :H])
                nc.sync.dma_start(xg[:, :, H:], src[:, :, H:])
            else:
                nc.sync.dma_start(xg[:], src)
            staged[gi] = xg

        held_big = None
        if n_held > 0:
            held_big = obh.tile([128, n_held, A], _F16, tag="held")

        def stage_compute_store(gi):
            row0, g = sched[gi]
            xg = staged.pop(gi)
            hold = hold0 <= row0 < main_tiles
            last = gi == n_groups - 1
            if hold:
                og = held_big[:, row0 - hold0 : row0 - hold0 + g, :]
            else:
                og = ob.tile([128, g, A], _F16, tag="ob" + str(g))
            if gi == 0 and g > 1 and w_split == 2:
                # k-half-major emission: both tiles' first-half matmuls run
                # off w's first half while its second half is still landing
                pox = []
                for _ in range(g):
                    p_t = po.tile([128, A], _F32, tag="po")
                    pox.append(p_t)
                for h in range(2):
                    for t in range(g):
                        for k in range(h * KC // 2, (h + 1) * KC // 2):
                            nc.tensor.matmul(
                                pox[t][:],
                                lhsT=xg[:, t, ts(k, 128)],
                                rhs=w_sb[:, k, :],
                                start=(k == 0),
                                stop=(k == KC - 1),
                            )
                for t in range(g):
                    nc.vector.tensor_add(og[:, t, :], pox[t][:], bias_sb[:])
                dst0 = out[bass.ds(row0 * 128, g * 128), :]
                dst0 = dst0.rearrange("(t p) a -> p t a", p=128)
                if not hold:
                    nc.scalar.dma_start(dst0, og[:])
                return
            for t in range(g):
                if last and t == g - 1:
                    # final tile: compute/store in column slices so the last
                    # store's issue latency overlaps the last matmuls and the
                    # final DVE add is short
                    ns = int(os.environ.get("K_LAST_SPLIT", "2"))
                    aw = A // ns
                    for h in range(ns):
                        p_out = poh.tile([128, A // 2], _F32, tag="poh")
                        for k in range(KC):
                            nc.tensor.matmul(
                                p_out[:, :aw],
                                lhsT=xg[:, t, ts(k, 128)],
                                rhs=w_sb[:, k, ts(h, aw)],
                                start=(k == 0),
                                stop=(k == KC - 1),
                            )
                        nc.vector.tensor_add(
                            og[:, t, ts(h, aw)], p_out[:, :aw], bias_sb[:, ts(h, aw)]
                        )
                    continue
                p_out = po.tile([128, A], _F32, tag="po")
                for k in range(KC):
                    nc.tensor.matmul(
                        p_out[:],
                        lhsT=xg[:, t, ts(k, 128)],
                        rhs=w_sb[:, k, :],
                        start=(k == 0),
                        stop=(k == KC - 1),
                    )
                # bias add fused with the f32 -> fp16 cast on copyback
                nc.vector.tensor_add(og[:, t, :], p_out[:], bias_sb[:])
            if hold:
                return  # flushed from held_big in the drain
            dst = out[bass.ds(row0 * 128, g * 128), :]
            if g > 1:
                dst = dst.rearrange("(t p) a -> p t a", p=128)
            else:
                dst = dst.rearrange("p (t a) -> p t a", t=1)
            in_drain = row0 >= n_tiles - tail
            if last or (in_drain and _GS_DRAIN_SP):
                nc.sync.dma_start(dst, og[:])
            else:
                nc.scalar.dma_start(dst, og[:])

        def flush_held():
            # flush the held-store region, chunked with a STRIDE across the
            # held tiles: every chunk contains one of the latest-computed
            # tiles, so no chunk's sem clears before the drain begins. This
            # keeps the flush transfers out of the input stream (the Tile
            # scheduler orders DMAs by readiness, not program order) and
            # saves them for the drain window, where they hide the last
            # tiles' compute latency.
            flush_eng = nc.sync if int(os.environ.get("K_FLUSH_SP", "0")) else nc.scalar
            n_chunks = max(1, (n_held + _FLUSH_CHUNK - 1) // _FLUSH_CHUNK)
            dst_all = out[bass.ds(hold0 * 128, n_held * 128), :]
            dst_all = dst_all.rearrange("(t p) a -> p t a", p=128)
            for c in range(n_chunks):
                flush_eng.dma_start(
                    dst_all[:, c::n_chunks, :], held_big[:, c::n_chunks, :]
                )

        for i in range(n_groups + _PIPE):
            if i < n_groups:
                stage_load(i)
            if i == n_groups - 1 and n_held > 0:
                # flushes sit after every load in program order so their
                # SemWaits never delay the drain loads' issue
                flush_held()
            if i >= _PIPE:
                stage_compute_store(i - _PIPE)

    nc.finalize()
    return nc


_NC_CACHE = None
LAST_RESULTS = None


def _get_nc():
    global _NC_CACHE
    if _NC_CACHE is None:
        _NC_CACHE = _build_nc()
    return _NC_CACHE


def _fold_weights(geodesic_weights: np.ndarray, W: np.ndarray) -> np.ndarray:
    """W' = W @ blockdiag(L(tanh(g))^T per 4-group), in float64."""
    q = np.tanh(geodesic_weights.astype(np.float64))[0]  # [N, 4]
    w_, i_, j_, k_ = q[:, 0], q[:, 1], q[:, 2], q[:, 3]
    n = q.shape[0]
    M = np.empty((n, 4, 4), dtype=np.float64)  # y_r = sum_s M[n, r, s] x_s
    M[:, 0] = np.stack([w_, -i_, -j_, -k_], axis=-1)
    M[:, 1] = np.stack([i_, w_, -k_, j_], axis=-1)
    M[:, 2] = np.stack([j_, k_, w_, -i_], axis=-1)
    M[:, 3] = np.stack([k_, -j_, i_, w_], axis=-1)
    W4 = W.astype(np.float64).reshape(A, n, 4)  # [a, n, r]
    Wp = np.einsum("anr,nrs->ans", W4, M).reshape(A, D)
    return Wp.astype(np.float32)  # [a, d]


def kernel(x, geodesic_weights, W, b, **_unused):
    x16 = np.asarray(x, dtype=np.float16)
    n_tiles = B_SHARD // 128
    # x_dev[core][t*128 + p, k*128 + b] = x16[core*B_SHARD + t*128 + b, k*128 + p]
    xs = x16.reshape(N_CORES, n_tiles, 128, KC, 128)  # [core, t, b, k, p]
    x_dev = np.ascontiguousarray(xs.transpose(0, 1, 4, 3, 2)).reshape(
        N_CORES, B_SHARD, KC * 128
    )

    Wp = _fold_weights(np.asarray(geodesic_weights), np.asarray(W))
    # device layout: w_dev[p, k*A + a] = Wp[a, 128k + p]
    w_dev = np.ascontiguousarray(
        Wp.T.reshape(KC, 128, A).transpose(1, 0, 2).reshape(128, KC * A)
    ).astype(np.float16)
    bias_dev = np.ascontiguousarray(
        np.broadcast_to(np.asarray(b, dtype=np.float16)[None, :], (128, A))
    )

    nc = _get_nc()
    in_maps = [
        {"x": x_dev[c], "w": w_dev, "bias": bias_dev} for c in range(N_CORES)
    ]
    res = run_bass_kernel_spmd(
        nc,
        in_maps,
        core_ids=list(range(N_CORES)),
        trace=bool(int(os.environ.get("KERNEL_TRACE", "0"))),
    )
    global LAST_RESULTS
    LAST_RESULTS = res
    out = np.concatenate([r["out"] for r in res.results], axis=0).astype(np.float32)
    return out
